# revision 9
# baseline (speedup 1.0000x reference)
"""LocalTopKCrossReadout Trainium2 kernel.

Data-parallel over batch: 8 cores, one batch element each, no collectives.

Per-core pipeline (feature-major layouts, d on partitions):
  FiLM (ctx -> gamma/beta) -> q_modT -> qT ; source -> srcT -> kT (feature-major)
  and v (row-major).  Per 128-query block (2 q_steps): windowed scores matmul
  (320 kv candidates), exp, top-32 via 4x(max8 + match_replace8 -> 0),
  dense weight matrix W = E - E_remaining, W^T via PE transpose,
  ctx = W @ V (dense matmul, no gather), scale by 1/sum(top32),
  out = ctx @ Wo, DMA out.

Precision: 'f32' exact everywhere, 'f32r' relaxed everywhere, or 'hybrid'
(selection path f32 / value path f32r).
"""
import os
import sys

sys.path.insert(0, "/opt/trn_rl_repo")

import numpy as np

from concourse import bacc, bass, mybir, tile
from concourse.bass_utils import run_bass_kernel_spmd

F32 = mybir.dt.float32
F32R = mybir.dt.float32r
Exp = mybir.ActivationFunctionType.Exp
Copy = mybir.ActivationFunctionType.Copy
Alu = mybir.AluOpType
AX = mybir.AxisListType

B, QS, QT, KS, KT, WIN, TOPK, DIM = 8, 16, 64, 64, 64, 2, 32, 512
LQ, LKV = QS * QT, KS * KT  # 1024, 4096
NCORES = 8
SCALE = 1.0 / float(np.sqrt(np.float32(DIM)))

CENTERS = np.round(np.linspace(0, KS - 1, QS)).astype(int)


SCW = 384  # score tile width: every window spans exactly 6 kv steps


def _win(step):
    """Fixed-width 384-token window, 128-aligned, within the step's kv half.

    Returns (w0 tokens, mask-col ranges [(a,b)...], real col span (off0,off1)).
    Real allowed region is [lo*64, (hi+1)*64); the rest is masked."""
    c = int(CENTERS[step])
    lo, hi = max(0, c - WIN), min(KS - 1, c + WIN)
    half_lo = (step // 8) * 32  # kv-step base of this query's half
    lo_e = min(lo - (lo % 2), half_lo + 32 - 6)
    assert lo_e >= half_lo
    off0, off1 = (lo - lo_e) * KT, (hi + 1 - lo_e) * KT
    masks = []
    if off0 > 0:
        masks.append((0, off0))
    if off1 < SCW:
        masks.append((off1, SCW))
    return lo_e * KT, masks, (off0, off1)


def build(prec="f32"):
    if prec == "f32":
        SEL, VAL = F32, F32
    elif prec == "f32r":
        SEL, VAL = F32R, F32R
    elif prec == "hybrid":
        SEL, VAL = F32, F32R
    else:
        raise ValueError(prec)

    nc = bacc.Bacc("TRN2", target_bir_lowering=False, debug=False,
                   num_devices=NCORES)

    query = nc.dram_tensor("query", [LQ, DIM], F32, kind="ExternalInput").ap()
    source = nc.dram_tensor("source", [LKV, DIM], F32, kind="ExternalInput").ap()
    ctx1 = nc.dram_tensor("ctx1", [1, DIM], F32, kind="ExternalInput").ap()
    ctx2 = nc.dram_tensor("ctx2", [1, DIM], F32, kind="ExternalInput").ap()
    Wq = nc.dram_tensor("Wq", [DIM, DIM], F32, kind="ExternalInput").ap()
    Wk = nc.dram_tensor("Wk", [DIM, DIM], F32, kind="ExternalInput").ap()
    Wv = nc.dram_tensor("Wv", [DIM, DIM], F32, kind="ExternalInput").ap()
    Wo = nc.dram_tensor("Wo", [DIM, DIM], F32, kind="ExternalInput").ap()
    Wc = nc.dram_tensor("Wc", [2 * DIM, 2 * DIM], F32, kind="ExternalInput").ap()
    bq = nc.dram_tensor("bq", [1, DIM], F32, kind="ExternalInput").ap()
    bk = nc.dram_tensor("bk", [1, DIM], F32, kind="ExternalInput").ap()
    bv = nc.dram_tensor("bv", [1, DIM], F32, kind="ExternalInput").ap()
    bo = nc.dram_tensor("bo", [1, DIM], F32, kind="ExternalInput").ap()
    bc = nc.dram_tensor("bc", [1, 2 * DIM], F32, kind="ExternalInput").ap()
    OUT = nc.dram_tensor("out", [LQ, DIM], F32, kind="ExternalOutput").ap()

    ident = nc.inline_tensor(np.eye(128, dtype=np.float32), name="ident").ap()

    def cast(ap, dt):
        return ap.bitcast(dt) if ap.dtype != dt else ap

    with tile.TileContext(nc) as tc:
        with (
            tc.tile_pool(name="persist", bufs=1) as pp,
            tc.tile_pool(name="persist2", bufs=1) as pp2,
        ):
            # ---- persistent constants / weights ----
            id_sel = pp.tile([128, 128], SEL)
            nc.sync.dma_start(id_sel[:], cast(ident, SEL))
            if VAL != SEL:
                id_val = pp.tile([128, 128], VAL)
                nc.sync.dma_start(id_val[:], cast(ident, VAL))
            else:
                id_val = id_sel

            def wload(dram, dt, name):
                t = pp.tile([128, 4, DIM], dt, name=name)
                nc.sync.dma_start(t[:], cast(
                    dram.rearrange("(c p) n -> p c n", p=128), dt))
                return t

            Wq_sb = wload(Wq, SEL, "Wq_sb")
            Wk_sb = wload(Wk, SEL, "Wk_sb")
            Wv_sb = wload(Wv, VAL, "Wv_sb")
            Wo_sb = wload(Wo, VAL, "Wo_sb")

            # column vectors [128, 4] (per-partition per-d-chunk scalars)
            def colvec(dram, n, dt, name, psum_pool, stage_pool):
                nchunk = n // 128
                st = stage_pool.tile([1, n], F32, tag="cv_stage")
                nc.sync.dma_start(st[:], dram[0:1, :])
                ps = psum_pool.tile([128, nchunk], F32, tag="cv_ps")
                for c in range(nchunk):
                    nc.tensor.transpose(ps[:, c:c + 1],
                                        st[0:1, c * 128:(c + 1) * 128],
                                        id_sel[0:1, 0:1])
                t = pp.tile([128, nchunk], dt, name=name)
                nc.vector.tensor_copy(t[:], ps[:])
                return t

            bo_sb = pp.tile([1, DIM], F32)
            nc.sync.dma_start(bo_sb[:], bo[0:1, :])
            bo_bc = pp.tile([128, DIM], F32)
            nc.gpsimd.partition_broadcast(bo_bc[:], bo_sb[0:1, :])

            # persistent activations
            qT_sb = pp2.tile([128, 4, LQ], SEL)           # 2 MB
            kT_sb = pp2.tile([128, 4, LKV // 2], SEL)     # 4 MB (per half)
            v_sb = pp2.tile([128, LKV // 2 // 128, DIM], VAL)  # 4 MB (per half)

            # ---- preproc: FiLM + qT ----
            with (
                tc.tile_pool(name="pre", bufs=1) as pre,
                tc.tile_pool(name="pre2", bufs=2) as pre2,
            ):
                with tc.tile_pool(name="film_ps", bufs=1, space="PSUM") as fps:
                    bqT = colvec(bq, DIM, F32, "bqT", fps, pre)
                    bkT = colvec(bk, DIM, F32, "bkT", fps, pre)
                    bvT = colvec(bv, DIM, F32, "bvT", fps, pre)

                    Wc_sb = pre.tile([128, 8, 2 * DIM], SEL)
                    nc.sync.dma_start(Wc_sb[:], cast(
                        Wc.rearrange("(c p) n -> p c n", p=128), SEL))
                    fstage = pre.tile([1, 2 * DIM], SEL)
                    nc.sync.dma_start(fstage[0:1, 0:DIM], cast(ctx1[0:1, :], SEL))
                    nc.sync.dma_start(fstage[0:1, DIM:2 * DIM],
                                      cast(ctx2[0:1, :], SEL))
                    fusedT_ps = fps.tile([128, 8], F32)
                    for c in range(8):
                        nc.tensor.transpose(fusedT_ps[:, c:c + 1],
                                            fstage[0:1, c * 128:(c + 1) * 128],
                                            id_sel[0:1, 0:1])
                    fusedT = pre.tile([128, 8], SEL)
                    nc.vector.tensor_copy(fusedT[:], fusedT_ps[:])

                    # gb = fused @ Wc + bc, row-major [1, 1024]
                    gb_ps = fps.tile([1, 2, DIM], F32)
                    for n in range(2):
                        for kc in range(8):
                            nc.tensor.matmul(
                                gb_ps[0:1, n, :],
                                fusedT[:, kc:kc + 1],
                                Wc_sb[:, kc, n * DIM:(n + 1) * DIM],
                                start=(kc == 0), stop=(kc == 7))
                    bc_sb = pre.tile([1, 2, DIM], F32)
                    nc.sync.dma_start(bc_sb[:],
                                      bc.rearrange("a (b c) -> a b c", b=2))
                    gb_sb = pre.tile([1, 2, DIM], SEL)
                    nc.vector.tensor_tensor(gb_sb[:], gb_ps[:], bc_sb[:],
                                            op=Alu.add)
                    gbT_ps = fps.tile([128, 8], F32)
                    for c in range(8):
                        nc.tensor.transpose(
                            gbT_ps[:, c:c + 1],
                            gb_sb[0:1, c // 4, (c % 4) * 128:(c % 4 + 1) * 128],
                            id_sel[0:1, 0:1])
                    gbT = pre.tile([128, 8], F32)
                    nc.vector.tensor_copy(gbT[:], gbT_ps[:])
                    gp1 = pre.tile([128, 4], F32)
                    nc.vector.tensor_scalar_add(gp1[:], gbT[:, 0:4], 1.0)

                # query -> q_modT (FiLM fused into transpose copy) -> qT
                q_modT = pre.tile([128, 4, LQ], SEL)
                with (
                    tc.tile_pool(name="q_ps", bufs=2, space="PSUM") as qps,
                    tc.tile_pool(name="qt_ps", bufs=2, space="PSUM") as qtps,
                ):
                    for i in range(LQ // 128):
                        qrow = pre2.tile([128, DIM], SEL, tag="qrow")
                        nc.sync.dma_start(
                            qrow[:], cast(query[i * 128:(i + 1) * 128, :], SEL))
                        qtr = qps.tile([128, 4, 128], F32, tag="qtr")
                        for c in range(4):
                            nc.tensor.transpose(
                                qtr[:, c, :], qrow[:, c * 128:(c + 1) * 128],
                                id_sel[:])
                        for c in range(4):
                            nc.vector.tensor_scalar(
                                q_modT[:, c, i * 128:(i + 1) * 128],
                                qtr[:, c, :], gp1[:, c:c + 1],
                                betaT_ap(gbT, c), op0=Alu.mult, op1=Alu.add)
                    for n in range(2):
                        for mc in range(4):
                            qTp = qtps.tile([128, DIM], F32, tag="qTp")
                            for kc in range(4):
                                nc.tensor.matmul(
                                    qTp[:],
                                    Wq_sb[:, kc, mc * 128:(mc + 1) * 128],
                                    q_modT[:, kc, n * DIM:(n + 1) * DIM],
                                    start=(kc == 0), stop=(kc == 3))
                            nc.vector.tensor_scalar(
                                qT_sb[:, mc, n * DIM:(n + 1) * DIM], qTp[:],
                                bqT[:, mc:mc + 1], None, op0=Alu.add)

            # ---- main: two kv halves ----
            for h in range(2):
                HB = h * (LKV // 2)  # half base token
                with (
                    tc.tile_pool(name="grp", bufs=2) as gsb,
                    tc.tile_pool(name="str_ps", bufs=2, space="PSUM") as pstr,
                    tc.tile_pool(name="kt_ps", bufs=2, space="PSUM") as pkt,
                    tc.tile_pool(name="v_ps", bufs=2, space="PSUM") as pv,
                ):
                    for g in range(4):
                        srcT = gsb.tile([128, 4, 512], SEL, tag="srcT")
                        if VAL != SEL:
                            srcTv = gsb.tile([128, 4, 512], VAL, tag="srcTv")
                        for r in range(4):
                            row = HB + g * 512 + r * 128
                            srow = gsb.tile([128, DIM], SEL, tag="srow")
                            nc.sync.dma_start(
                                srow[:], cast(source[row:row + 128, :], SEL))
                            strp = pstr.tile([128, 4, 128], F32, tag="strp")
                            for c in range(4):
                                nc.tensor.transpose(
                                    strp[:, c, :],
                                    srow[:, c * 128:(c + 1) * 128], id_sel[:])
                            nc.scalar.activation(
                                srcT[:, :, r * 128:(r + 1) * 128], strp[:], Copy)
                            if VAL != SEL:
                                nc.scalar.activation(
                                    srcTv[:, :, r * 128:(r + 1) * 128], strp[:],
                                    Copy)
                        srcT_v = srcTv if VAL != SEL else srcT
                        for mc in range(4):
                            kTp = pkt.tile([128, DIM], F32, tag="kTp")
                            for kc in range(4):
                                nc.tensor.matmul(
                                    kTp[:],
                                    Wk_sb[:, kc, mc * 128:(mc + 1) * 128],
                                    srcT[:, kc, :],
                                    start=(kc == 0), stop=(kc == 3))
                            nc.vector.tensor_scalar(
                                kT_sb[:, mc, g * 512:(g + 1) * 512], kTp[:],
                                bkT[:, mc:mc + 1], None, op0=Alu.add)
                        for r in range(4):
                            vp = pv.tile([128, DIM], F32, tag="vp")
                            for kc in range(4):
                                nc.tensor.matmul(
                                    vp[:],
                                    srcT_v[:, kc, r * 128:(r + 1) * 128],
                                    Wv_sb[:, kc, :],
                                    start=(kc == 0), stop=(kc == 3))
                            nc.scalar.activation(v_sb[:, g * 4 + r, :], vp[:],
                                                 Copy)

                with (
                    tc.tile_pool(name="blk", bufs=2) as bsb,
                    tc.tile_pool(name="sc_ps", bufs=1, space="PSUM") as psc,
                    tc.tile_pool(name="wt_ps", bufs=1, space="PSUM") as pwt,
                    tc.tile_pool(name="cx_ps", bufs=1, space="PSUM") as pcx,
                    tc.tile_pool(name="ct_ps", bufs=1, space="PSUM") as pct,
                    tc.tile_pool(name="o_ps", bufs=2, space="PSUM") as pout,
                ):
                    for b in range(4):
                        BLK = h * 4 + b
                        steps = (2 * BLK, 2 * BLK + 1)
                        wins = [_win(s) for s in steps]

                        # per half-step [64, SCW] psum so PE outputs stay at
                        # partition base 0 (base-64 outputs crash this stack)
                        scps = [psc.tile([64, SCW], F32, tag="scp0", name="scp0"),
                                psc.tile([64, SCW], F32, tag="scp1", name="scp1")]
                        E = bsb.tile([128, SCW], F32, tag="E")
                        for j, (w0, masks, (off0, off1)) in enumerate(wins):
                            for kc in range(4):
                                nc.tensor.matmul(
                                    scps[j][0:64, off0:off1],
                                    qT_sb[:, kc,
                                          steps[j] * 64:(steps[j] + 1) * 64],
                                    kT_sb[:, kc,
                                          w0 - HB + off0:w0 - HB + off1],
                                    start=(kc == 0), stop=(kc == 3))
                            for (a, bb) in masks:
                                nc.vector.memset(scps[j][0:64, a:bb], -1e4)
                            nc.scalar.activation(E[j * 64:(j + 1) * 64, :],
                                                 scps[j][:], Exp, scale=SCALE)

                        top8 = bsb.tile([128, 32], F32, tag="top8")
                        Ea = bsb.tile([128, SCW], F32, tag="Ea")
                        Eb = bsb.tile([128, SCW], F32, tag="Eb")
                        cur = E
                        for r in range(4):
                            nc.vector.max(top8[:, r * 8:(r + 1) * 8], cur[:])
                            nxt = Ea if r % 2 == 0 else Eb
                            nc.vector.match_replace(
                                nxt[:], top8[:, r * 8:(r + 1) * 8], cur[:], 0.0)
                            cur = nxt
                        W = bsb.tile([128, SCW], VAL, tag="W")
                        nc.vector.tensor_sub(W[:], E[:], cur[:])
                        den = bsb.tile([128, 1], F32, tag="den")
                        nc.vector.reduce_sum(den[:], top8[:], axis=AX.X)
                        rec = bsb.tile([128, 1], F32, tag="rec")
                        nc.vector.reciprocal(rec[:], den[:])

                        # W^T via full 128-wide PE transposes (base 0 only)
                        wtp = pwt.tile([128, SCW], F32, tag="wtp")
                        for pi in range(3):
                            nc.tensor.transpose(
                                wtp[:, pi * 128:(pi + 1) * 128],
                                W[:, pi * 128:(pi + 1) * 128], id_val[:])
                        wt = bsb.tile([128, SCW], VAL, tag="wt")
                        nc.vector.tensor_copy(wt[:], wtp[:])

                        cxps = [pcx.tile([64, DIM], F32, tag="cxp0", name="cxp0"),
                                pcx.tile([64, DIM], F32, tag="cxp1", name="cxp1")]
                        csc = bsb.tile([128, DIM], VAL, tag="csc")
                        for j, (w0, _m, _s) in enumerate(wins):
                            vt0 = (w0 - HB) // 128
                            for pi in range(3):
                                nc.tensor.matmul(
                                    cxps[j][0:64, :],
                                    wt[:, pi * 128 + j * 64:
                                       pi * 128 + j * 64 + 64],
                                    v_sb[:, vt0 + pi, :],
                                    start=(pi == 0), stop=(pi == 2))
                            nc.vector.tensor_scalar(
                                csc[j * 64:(j + 1) * 64, :], cxps[j][:],
                                rec[j * 64:(j + 1) * 64, :], None,
                                op0=Alu.mult)

                        ctp = pct.tile([128, 4, 128], F32, tag="ctp")
                        for c in range(4):
                            nc.tensor.transpose(
                                ctp[:, c, :], csc[:, c * 128:(c + 1) * 128],
                                id_val[:])
                        ctxT = bsb.tile([128, 4, 128], VAL, tag="ctxT")
                        for c in range(4):
                            nc.vector.tensor_scalar(
                                ctxT[:, c, :], ctp[:, c, :], bvT[:, c:c + 1],
                                None, op0=Alu.add)

                        outp = pout.tile([128, DIM], F32, tag="outp")
                        for kc in range(4):
                            nc.tensor.matmul(outp[:], ctxT[:, kc, :],
                                             Wo_sb[:, kc, :],
                                             start=(kc == 0), stop=(kc == 3))
                        osb = bsb.tile([128, DIM], F32, tag="osb")
                        nc.vector.tensor_tensor(osb[:], outp[:], bo_bc[:],
                                                op=Alu.add)
                        nc.sync.dma_start(OUT[BLK * 128:(BLK + 1) * 128, :],
                                          osb[:])

    nc.compile()
    return nc


def betaT_ap(gbT, c):
    return gbT[:, 4 + c:5 + c]


_CACHE = {}


def get_nc(prec):
    if prec not in _CACHE:
        _CACHE[prec] = build(prec)
    return _CACHE[prec]


def _run(inputs, prec=None, trace=False, **kw):
    prec = prec or os.environ.get("KERNEL_PREC", "f32")
    nc = get_nc(prec)
    f = lambda x: np.ascontiguousarray(np.asarray(x, dtype=np.float32))
    query = f(inputs["query"]).reshape(B, LQ, DIM)
    source = f(inputs["source"]).reshape(B, LKV, DIM)
    in_maps = []
    for i in range(NCORES):
        m = {
            "query": query[i], "source": source[i],
            "ctx1": f(inputs["ctx1"])[i:i + 1], "ctx2": f(inputs["ctx2"])[i:i + 1],
            "Wq": f(inputs["Wq"]), "Wk": f(inputs["Wk"]),
            "Wv": f(inputs["Wv"]), "Wo": f(inputs["Wo"]), "Wc": f(inputs["Wc"]),
            "bq": f(inputs["bq"]).reshape(1, DIM),
            "bk": f(inputs["bk"]).reshape(1, DIM),
            "bv": f(inputs["bv"]).reshape(1, DIM),
            "bo": f(inputs["bo"]).reshape(1, DIM),
            "bc": f(inputs["bc"]).reshape(1, 2 * DIM),
        }
        in_maps.append(m)
    res = run_bass_kernel_spmd(nc, in_maps, list(range(NCORES)), trace=trace,
                               **kw)
    out = np.stack([res.results[i]["out"] for i in range(NCORES)])
    return out.reshape(B, QS, QT, DIM), res


def kernel(**inputs):
    out, _ = _run(inputs)
    return out
